# revision 10
# baseline (speedup 1.0000x reference)
"""BQQ linear inference kernel for 8 Trainium2 NeuronCores.

Factorized form: after (host-side) activation quantization the op is linear
in X_int, and each (j,k) weight block is rank-32 plus rank-2 corrections:

  out[b, (j,m)] = act_scale * sum_k [ Yc_jk^T (Z_jk X_k^T) ]{m,b}  (main)
               + corrections(Sx, out3, out4) + bias

Device kernel per core (j-sharded, 4 of 32 j-blocks):
  stage0  corrections+bias enter PSUM as one 37-row matmul per j block
          (rows: 32 Sx terms, 4 per-j scalars, 1 bias row).
  stage1  T_k[(j,p,l), b] = Zstack_k^T @ X_k^T   (one 128x512 MM per k)
  copy    T_k PSUM -> SBUF fp16, split across vector/scalar engines
  stage2  outp_j[m, b] += Yc_jk^T @ T_k[32j:32j+32]  (4 row-tiled rank-32
          MMs per k, tile_position=(32j,0), software-pipelined one k behind
          stage1 so the PE never waits on the copies)
  out     PSUM -> SBUF bf16, DMA out as [jm, b]; host transposes.

x ships int8 (2MB) and is upcast to bf16 on the otherwise idle gpsimd
engine.  Per-core HBM traffic ~5.6 MB; PE streams ~half the dense GEMM.
"""

import numpy as np
import ml_dtypes

import concourse.bass as bass
import concourse.bacc as bacc
import concourse.mybir as mybir
from concourse.tile import TileContext
from concourse.tile_rust import add_dep_helper
from concourse.bass_utils import run_bass_kernel_spmd

F32 = mybir.dt.float32
BF16 = mybir.dt.bfloat16
FP16 = mybir.dt.float16
I8 = mybir.dt.int8

P_, J, K, M, L, N = 2, 32, 32, 128, 16, 128
B = 512                  # tokens
NCORES = 8
JLOC = J // NCORES       # 4 j-blocks per core
CPJ = JLOC * M           # 512 output rows per core ((j,m) major)
QMAX = 127.0
UROWS = 37               # 32 Sx rows + 4 corr rows + 1 bias row
CHUNKS = [1, 1, 2, 4, 4, 4, 4, 4, 4, 4]
WARMUP = 64

_CACHE = {}


def _build_bass():
    nc = bacc.Bacc()
    xt_d = nc.declare_dram_parameter("xt8", [N, K * B], I8, isOutput=False)
    z_d = nc.declare_dram_parameter("zt", [N, K * 128], BF16, isOutput=False)
    y_d = nc.declare_dram_parameter("yt", [128, K * M], FP16, isOutput=False)
    u_d = nc.declare_dram_parameter("ut", [UROWS, JLOC * M], FP16,
                                    isOutput=False)
    r_d = nc.declare_dram_parameter("rt", [UROWS, B], FP16, isOutput=False)
    out_d = nc.declare_dram_parameter("out", [CPJ, B], BF16, isOutput=True)

    with TileContext(nc) as tc:
        with tc.tile_pool(name="big", bufs=1) as big, \
             tc.tile_pool(name="sm", bufs=1) as sm, \
             tc.tile_pool(name="ot", bufs=4) as ot, \
             tc.tile_pool(name="psum", bufs=1, space="PSUM") as pp:
            xi8 = big.tile([N, K * B], I8)        # x^T int8
            xbt = big.tile([N, K * B], BF16)      # x^T upcast to bf16
            ztt = big.tile([N, K * 128], BF16)    # Z sign stacks
            ytt = big.tile([128, K * M], FP16)    # scaled Y stacks
            tsb = [big.tile([128, B], FP16, name=f'tsb{i}') for i in range(2)]
            utt = sm.tile([UROWS, JLOC * M], FP16)
            rtt = sm.tile([UROWS, B], FP16)
            wz = sm.tile([128, 192], BF16)        # zeros for PE warmup
            nc.vector.memset(wz[:], 0.0)

            tp = [pp.tile([128, B], F32, name=f"tps{i}", tag=f"tps{i}")
                  for i in range(2)]
            outp = [pp.tile([128, B], F32, name=f"op{i}", tag=f"op{i}")
                    for i in range(4)]
            wps = pp.tile([128, 64], F32, name="wps", tag="wps")

            # Phase A: k-ordered streaming.  x^T int8 on the sync HWDGE
            # ring; Z/Y stacks on the scalar ring; correction matrices
            # first (they open the PSUM accumulation groups).  Slim dummy
            # matmuls paced by the first DMA warm the PE HAM clock.
            nc.sync.dma_start(out=utt[:], in_=u_d[:])
            nc.sync.dma_start(out=rtt[:], in_=r_d[:])
            k0 = 0
            for ci, nk in enumerate(CHUNKS):
                dma = nc.sync.dma_start(out=xi8[:, k0 * B:(k0 + nk) * B],
                                        in_=xt_d[:, k0 * B:(k0 + nk) * B])
                nc.scalar.dma_start(out=ztt[:, k0 * 128:(k0 + nk) * 128],
                                    in_=z_d[:, k0 * 128:(k0 + nk) * 128])
                nc.scalar.dma_start(out=ytt[:, k0 * M:(k0 + nk) * M],
                                    in_=y_d[:, k0 * M:(k0 + nk) * M])
                if ci == 0:
                    for w in range(WARMUP):
                        mm = nc.tensor.matmul(
                            wps[:], lhsT=wz[:, 0:128],
                            rhs=wz[:, 128:192], start=True, stop=True)
                        add_dep_helper(mm.ins, dma.ins,
                                       reason="pace PE warmup with x DMA")
                # upcast on gpsimd in <=2-k pieces (keeps it off the
                # vector/scalar engines, which carry the T copies)
                g = k0
                while g < k0 + nk:
                    ng = min(2, k0 + nk - g)
                    nc.gpsimd.tensor_copy(out=xbt[:, g * B:(g + ng) * B],
                                          in_=xi8[:, g * B:(g + ng) * B])
                    g += ng
                k0 += nk

            # Phase B: corrections open the output accumulation groups.
            for j in range(JLOC):
                nc.tensor.matmul(
                    outp[j][:], lhsT=utt[:, j * M:(j + 1) * M], rhs=rtt[:],
                    start=True, stop=False)

            def s1(k):
                nc.tensor.matmul(
                    tp[k % 2][:], lhsT=ztt[:, k * 128:(k + 1) * 128],
                    rhs=xbt[:, k * B:(k + 1) * B], start=True, stop=True)

            def tcopy(k):
                t = tsb[k % 2]
                nc.vector.tensor_copy(out=t[:, 0:256], in_=tp[k % 2][:, 0:256])
                nc.scalar.copy(t[:, 256:512], tp[k % 2][:, 256:512])

            def s2(k):
                for j in range(JLOC):
                    nc.tensor.matmul(
                        outp[j][:],
                        lhsT=ytt[32 * j:32 * (j + 1), k * M:(k + 1) * M],
                        rhs=tsb[k % 2][32 * j:32 * (j + 1), :],
                        start=False, stop=(k == K - 1),
                        tile_position=(32 * j, 0))

            s1(0)
            tcopy(0)
            for k in range(1, K):
                s1(k)
                s2(k - 1)
                tcopy(k)
            s2(K - 1)

            # Phase C: PSUM -> SBUF bf16 (split engines), DMA out rows
            # (j,m); the host transposes to [b, (j,m)].
            for j in range(JLOC):
                o = ot.tile([128, B], BF16)
                if j % 2 == 0:
                    nc.scalar.copy(o[:], outp[j][:])
                else:
                    nc.vector.tensor_copy(out=o[:], in_=outp[j][:])
                eng = nc.sync if j % 2 == 0 else nc.scalar
                eng.dma_start(out=out_d[j * 128:(j + 1) * 128, :], in_=o[:])
    return nc


def _prepare(inputs):
    x = np.asarray(inputs["input"], dtype=np.float32)
    Ys = np.asarray(inputs["Y_sign"], np.float32)
    Zs = np.asarray(inputs["Z_sign"], np.float32)
    ysc = np.asarray(inputs["Y_scale"], np.float32)[..., 0, 0]
    zsc = np.asarray(inputs["Z_scale"], np.float32)[..., 0, 0]
    A = np.asarray(inputs["A"], np.float32)
    bias = np.asarray(inputs["bias"], np.float32)
    a0, a1, a2, a3 = A[..., 0], A[..., 1], A[..., 2], A[..., 3]

    # activation quantization on host (exact global max/min, RNE round)
    act_scale = max((float(x.max()) - float(x.min())) / (2.0 * QMAX), 1e-8)
    Xi = np.clip(np.round(x.reshape(B, K * N) / act_scale), -QMAX, QMAX)
    Xkn = Xi.reshape(B, K, N)
    Sx = Xkn.sum(-1)                                   # [B,K] (exact ints)

    c = a0 * ysc * zsc * act_scale                     # [P,J,K]
    B2 = np.einsum('pjk,pjkm->jkm', a1 * ysc, Ys.sum(-1)) * act_scale
    C3 = np.einsum('pjk,pjkn->jkn', a2 * zsc, Zs.sum(-2))
    out3 = np.einsum('bkn,jkn->bj', Xkn, C3) * act_scale
    out4 = (Sx @ a3.sum(0).T) * act_scale              # [B,J]
    corr34 = out3 + out4

    xt8 = np.ascontiguousarray(
        Xi.reshape(B, K, N).transpose(2, 1, 0).reshape(N, K * B)).astype(
            np.int8)

    in_maps = []
    for cid in range(NCORES):
        jsl = slice(cid * JLOC, (cid + 1) * JLOC)
        zt = np.ascontiguousarray(
            Zs[:, jsl].transpose(4, 2, 1, 0, 3).reshape(N, K * 128)).astype(
                ml_dtypes.bfloat16)                    # [n,(k,j,p,l)]
        yt = np.ascontiguousarray(
            (c[:, jsl, :, None, None] * Ys[:, jsl]).transpose(
                1, 0, 4, 2, 3).reshape(128, K * M)).astype(np.float16)
        ut = np.zeros((UROWS, JLOC * M), np.float16)
        rt = np.zeros((UROWS, B), np.float16)
        for j in range(JLOC):
            jg = cid * JLOC + j
            ut[0:32, j * M:(j + 1) * M] = B2[jg].astype(np.float16)
            ut[32 + j, j * M:(j + 1) * M] = 1.0
            ut[36, j * M:(j + 1) * M] = bias[jg * M:(jg + 1) * M].astype(
                np.float16)
            rt[32 + j] = corr34[:, jg].astype(np.float16)
        rt[0:32] = Sx.T.astype(np.float16)
        rt[36] = 1.0
        in_maps.append({"xt8": xt8, "zt": zt, "yt": yt,
                        "ut": ut, "rt": rt})
    return in_maps


def _run(inputs, trace=False):
    if "nc" not in _CACHE:
        nc = _build_bass()
        nc.finalize()          # run bacc passes (reg alloc, wait splitting)
        _CACHE["nc"] = nc
    nc = _CACHE["nc"]
    in_maps = _prepare(inputs)
    res = run_bass_kernel_spmd(nc, in_maps, list(range(NCORES)), trace=trace)
    out = np.concatenate(
        [res.results[c]["out"].astype(np.float32).T for c in range(NCORES)],
        axis=1)
    out = out.reshape(1, B, J * M)
    return out, res


def kernel(**inputs) -> np.ndarray:
    out, _ = _run(inputs, trace=False)
    return out


# revision 14
# speedup vs baseline: 1.2410x; 1.2410x over previous
"""BQQ linear inference kernel for 8 Trainium2 NeuronCores.

Factorized form: after (host-side) activation quantization the op is linear
in X_int, and each (j,k) weight block is rank-32 plus rank-2 corrections:

  out[b, (j,m)] = act_scale * sum_k [ Yc_jk^T (Z_jk X_k^T) ]{m,b}  (main)
               + corrections(Sx, out3, out4) + bias

Device kernel per core (j-sharded, 4 of 32 j-blocks):
  stage0  corrections+bias enter PSUM as one 37-row matmul per j block
          (rows: 32 Sx terms, 4 per-j scalars, 1 bias row).
  stage1  T_k[(j,p,l), b] = Zstack_k^T @ X_k^T   (one 128x512 MM per k)
  copy    T_k PSUM -> SBUF fp16, split across vector/scalar engines
  stage2  outp_j[m, b] += Yc_jk^T @ T_k[32j:32j+32]  (4 row-tiled rank-32
          MMs per k, tile_position=(32j,0), software-pipelined one k behind
          stage1 so the PE never waits on the copies)
  out     PSUM -> SBUF bf16, DMA out as [jm, b]; host transposes.

x ships int8 (2MB) and is upcast to bf16 on the otherwise idle gpsimd
engine.  Per-core HBM traffic ~5.6 MB; PE streams ~half the dense GEMM.
"""

import numpy as np
import ml_dtypes

import concourse.bass as bass
import concourse.bacc as bacc
import concourse.mybir as mybir
from concourse.tile import TileContext
from concourse.tile_rust import add_dep_helper
from concourse.bass_utils import run_bass_kernel_spmd

F32 = mybir.dt.float32
BF16 = mybir.dt.bfloat16
FP16 = mybir.dt.float16
I8 = mybir.dt.int8

P_, J, K, M, L, N = 2, 32, 32, 128, 16, 128
B = 512                  # tokens
NCORES = 8
JLOC = J // NCORES       # 4 j-blocks per core
CPJ = JLOC * M           # 512 output rows per core ((j,m) major)
QMAX = 127.0
UROWS = 37               # 32 Sx rows + 4 corr rows + 1 bias row
CHUNKS = [1, 1, 2, 4, 4, 4, 4, 4, 4, 4]
WARMUP = 64

_CACHE = {}


def _build_bass():
    nc = bacc.Bacc()
    xt_d = nc.declare_dram_parameter("xt8", [N, K * B], I8, isOutput=False)
    z_d = nc.declare_dram_parameter("zt", [N, K * 128], BF16, isOutput=False)
    y_d = nc.declare_dram_parameter("yt", [128, K * M], FP16, isOutput=False)
    u_d = nc.declare_dram_parameter("ut", [UROWS, JLOC * M], FP16,
                                    isOutput=False)
    r_d = nc.declare_dram_parameter("rt", [UROWS, B], FP16, isOutput=False)
    out_d = nc.declare_dram_parameter("out", [CPJ, B], BF16, isOutput=True)

    with TileContext(nc) as tc:
        with tc.tile_pool(name="big", bufs=1) as big, \
             tc.tile_pool(name="sm", bufs=1) as sm, \
             tc.tile_pool(name="ot", bufs=4) as ot, \
             tc.tile_pool(name="psum", bufs=1, space="PSUM") as pp:
            xi8 = big.tile([N, K * B], I8)        # x^T int8
            xbt = big.tile([N, K * B], BF16)      # x^T upcast to bf16
            ztt = big.tile([N, K * 128], BF16)    # Z sign stacks
            ytt = big.tile([128, K * M], FP16)    # scaled Y stacks
            tsb = [big.tile([128, B], FP16, name=f'tsb{i}') for i in range(2)]
            utt = sm.tile([UROWS, JLOC * M], FP16)
            rtt = sm.tile([UROWS, B], FP16)
            wz = sm.tile([128, 192], BF16)        # zeros for PE warmup
            nc.vector.memset(wz[:], 0.0)

            tp = [pp.tile([128, B], F32, name=f"tps{i}", tag=f"tps{i}")
                  for i in range(2)]
            outp = [pp.tile([128, B], F32, name=f"op{i}", tag=f"op{i}")
                    for i in range(4)]
            wps = pp.tile([128, 64], F32, name="wps", tag="wps")

            # Phase A: k-ordered streaming.  x^T int8 on the sync HWDGE
            # ring; Z/Y stacks on the scalar ring; correction matrices
            # first (they open the PSUM accumulation groups).  Slim dummy
            # matmuls paced by the first DMA warm the PE HAM clock.
            nc.sync.dma_start(out=utt[:], in_=u_d[:])
            nc.sync.dma_start(out=rtt[:], in_=r_d[:])
            k0 = 0
            for ci, nk in enumerate(CHUNKS):
                dma = nc.sync.dma_start(out=xi8[:, k0 * B:(k0 + nk) * B],
                                        in_=xt_d[:, k0 * B:(k0 + nk) * B])
                nc.scalar.dma_start(out=ztt[:, k0 * 128:(k0 + nk) * 128],
                                    in_=z_d[:, k0 * 128:(k0 + nk) * 128])
                nc.scalar.dma_start(out=ytt[:, k0 * M:(k0 + nk) * M],
                                    in_=y_d[:, k0 * M:(k0 + nk) * M])
                if ci == 0:
                    for w in range(WARMUP):
                        mm = nc.tensor.matmul(
                            wps[:], lhsT=wz[:, 0:128],
                            rhs=wz[:, 128:192], start=True, stop=True)
                        add_dep_helper(mm.ins, dma.ins,
                                       reason="pace PE warmup with x DMA")
                k0 += nk

            # x upcast int8 -> bf16 on DVE (the scalar engine faults on
            # int8 input; gpsimd is ~10x too slow)
            def upcast(k):
                nc.vector.tensor_copy(out=xbt[:, k * B:(k + 1) * B],
                                      in_=xi8[:, k * B:(k + 1) * B])

            UPLOOK = 6           # upcast lookahead inside the main k loop
            for k in range(UPLOOK):
                upcast(k)

            # Phase B: corrections open the output accumulation groups.
            for j in range(JLOC):
                nc.tensor.matmul(
                    outp[j][:], lhsT=utt[:, j * M:(j + 1) * M], rhs=rtt[:],
                    start=True, stop=False)

            def s1(k):
                nc.tensor.matmul(
                    tp[k % 2][:], lhsT=ztt[:, k * 128:(k + 1) * 128],
                    rhs=xbt[:, k * B:(k + 1) * B], start=True, stop=True)

            def tcopy(k):
                t = tsb[k % 2]
                nc.vector.tensor_copy(out=t[:, 0:128], in_=tp[k % 2][:, 0:128])
                nc.scalar.copy(t[:, 128:512], tp[k % 2][:, 128:512])

            def s2(k):
                for j in range(JLOC):
                    nc.tensor.matmul(
                        outp[j][:],
                        lhsT=ytt[32 * j:32 * (j + 1), k * M:(k + 1) * M],
                        rhs=tsb[k % 2][32 * j:32 * (j + 1), :],
                        start=False, stop=(k == K - 1),
                        tile_position=(32 * j, 0))

            s1(0)
            tcopy(0)
            for k in range(1, K):
                s1(k)
                s2(k - 1)
                tcopy(k)
                if k - 1 + UPLOOK < K:
                    upcast(k - 1 + UPLOOK)
            s2(K - 1)

            # Phase C: PSUM -> SBUF bf16 (split engines), DMA out rows
            # (j,m); the host transposes to [b, (j,m)].
            for j in range(JLOC):
                o = ot.tile([128, B], BF16)
                if j % 2 == 0:
                    nc.scalar.copy(o[:], outp[j][:])
                else:
                    nc.vector.tensor_copy(out=o[:], in_=outp[j][:])
                eng = nc.sync if j % 2 == 0 else nc.scalar
                eng.dma_start(out=out_d[j * 128:(j + 1) * 128, :], in_=o[:])
    return nc


def _prepare(inputs):
    x = np.asarray(inputs["input"], dtype=np.float32)
    Ys = np.asarray(inputs["Y_sign"], np.float32)
    Zs = np.asarray(inputs["Z_sign"], np.float32)
    ysc = np.asarray(inputs["Y_scale"], np.float32)[..., 0, 0]
    zsc = np.asarray(inputs["Z_scale"], np.float32)[..., 0, 0]
    A = np.asarray(inputs["A"], np.float32)
    bias = np.asarray(inputs["bias"], np.float32)
    a0, a1, a2, a3 = A[..., 0], A[..., 1], A[..., 2], A[..., 3]

    # activation quantization on host (exact global max/min, RNE round)
    act_scale = max((float(x.max()) - float(x.min())) / (2.0 * QMAX), 1e-8)
    Xi = np.clip(np.round(x.reshape(B, K * N) / act_scale), -QMAX, QMAX)
    Xkn = Xi.reshape(B, K, N)
    Sx = Xkn.sum(-1)                                   # [B,K] (exact ints)

    c = a0 * ysc * zsc * act_scale                     # [P,J,K]
    B2 = np.einsum('pjk,pjkm->jkm', a1 * ysc, Ys.sum(-1)) * act_scale
    C3 = np.einsum('pjk,pjkn->jkn', a2 * zsc, Zs.sum(-2))
    out3 = np.einsum('bkn,jkn->bj', Xkn, C3) * act_scale
    out4 = (Sx @ a3.sum(0).T) * act_scale              # [B,J]
    corr34 = out3 + out4

    xt8 = np.ascontiguousarray(
        Xi.reshape(B, K, N).transpose(2, 1, 0).reshape(N, K * B)).astype(
            np.int8)

    in_maps = []
    for cid in range(NCORES):
        jsl = slice(cid * JLOC, (cid + 1) * JLOC)
        zt = np.ascontiguousarray(
            Zs[:, jsl].transpose(4, 2, 1, 0, 3).reshape(N, K * 128)).astype(
                ml_dtypes.bfloat16)                    # [n,(k,j,p,l)]
        yt = np.ascontiguousarray(
            (c[:, jsl, :, None, None] * Ys[:, jsl]).transpose(
                1, 0, 4, 2, 3).reshape(128, K * M)).astype(np.float16)
        ut = np.zeros((UROWS, JLOC * M), np.float16)
        rt = np.zeros((UROWS, B), np.float16)
        for j in range(JLOC):
            jg = cid * JLOC + j
            ut[0:32, j * M:(j + 1) * M] = B2[jg].astype(np.float16)
            ut[32 + j, j * M:(j + 1) * M] = 1.0
            ut[36, j * M:(j + 1) * M] = bias[jg * M:(jg + 1) * M].astype(
                np.float16)
            rt[32 + j] = corr34[:, jg].astype(np.float16)
        rt[0:32] = Sx.T.astype(np.float16)
        rt[36] = 1.0
        in_maps.append({"xt8": xt8, "zt": zt, "yt": yt,
                        "ut": ut, "rt": rt})
    return in_maps


def _run(inputs, trace=False):
    if "nc" not in _CACHE:
        nc = _build_bass()
        nc.finalize()          # run bacc passes (reg alloc, wait splitting)
        _CACHE["nc"] = nc
    nc = _CACHE["nc"]
    in_maps = _prepare(inputs)
    res = run_bass_kernel_spmd(nc, in_maps, list(range(NCORES)), trace=trace)
    out = np.concatenate(
        [res.results[c]["out"].astype(np.float32).T for c in range(NCORES)],
        axis=1)
    out = out.reshape(1, B, J * M)
    return out, res


def kernel(**inputs) -> np.ndarray:
    out, _ = _run(inputs, trace=False)
    return out


# revision 15
# speedup vs baseline: 1.4165x; 1.1414x over previous
"""BQQ linear inference kernel for 8 Trainium2 NeuronCores.

Factorized form: after (host-side) activation quantization the op is linear
in X_int, and each (j,k) weight block is rank-32 plus rank-2 corrections:

  out[b, (j,m)] = act_scale * sum_k [ Yc_jk^T (Z_jk X_k^T) ]{m,b}  (main)
               + corrections(Sx, out3, out4) + bias

Device kernel per core (j-sharded, 4 of 32 j-blocks):
  stage0  corrections+bias enter PSUM as one 37-row matmul per j block
          (rows: 32 Sx terms, 4 per-j scalars, 1 bias row).
  stage1  T_k[(j,p,l), b] = Zstack_k^T @ X_k^T   (one 128x512 MM per k)
  copy    T_k PSUM -> SBUF fp16, split across vector/scalar engines
  stage2  outp_j[m, b] += Yc_jk^T @ T_k[32j:32j+32]  (4 row-tiled rank-32
          MMs per k, tile_position=(32j,0), software-pipelined one k behind
          stage1 so the PE never waits on the copies)
  out     PSUM -> SBUF bf16, DMA out as [jm, b]; host transposes.

x ships int8 (2MB) and is upcast to bf16 on the otherwise idle gpsimd
engine.  Per-core HBM traffic ~5.6 MB; PE streams ~half the dense GEMM.
"""

import numpy as np
import ml_dtypes

import concourse.bass as bass
import concourse.bacc as bacc
import concourse.mybir as mybir
from concourse.tile import TileContext
from concourse.tile_rust import add_dep_helper
from concourse.bass_utils import run_bass_kernel_spmd

F32 = mybir.dt.float32
BF16 = mybir.dt.bfloat16
FP16 = mybir.dt.float16
I8 = mybir.dt.int8

P_, J, K, M, L, N = 2, 32, 32, 128, 16, 128
B = 512                  # tokens
NCORES = 8
JLOC = J // NCORES       # 4 j-blocks per core
CPJ = JLOC * M           # 512 output rows per core ((j,m) major)
QMAX = 127.0
UROWS = 37               # 32 Sx rows + 4 corr rows + 1 bias row
CHUNKS = [1, 1, 2, 4, 4, 4, 4, 4, 4, 4]
WARMUP = 64

_CACHE = {}


def _build_bass():
    nc = bacc.Bacc()
    xt_d = nc.declare_dram_parameter("xt8", [N, K * B], I8, isOutput=False)
    z_d = nc.declare_dram_parameter("zt", [N, K * 128], BF16, isOutput=False)
    y_d = nc.declare_dram_parameter("yt", [128, K * M], FP16, isOutput=False)
    u_d = nc.declare_dram_parameter("ut", [UROWS, JLOC * M], FP16,
                                    isOutput=False)
    r_d = nc.declare_dram_parameter("rt", [UROWS, B], FP16, isOutput=False)
    out_d = nc.declare_dram_parameter("out", [CPJ, B], BF16, isOutput=True)

    with TileContext(nc) as tc:
        with tc.tile_pool(name="big", bufs=1) as big, \
             tc.tile_pool(name="sm", bufs=1) as sm, \
             tc.tile_pool(name="ot", bufs=4) as ot, \
             tc.tile_pool(name="psum", bufs=1, space="PSUM") as pp:
            xi8 = big.tile([N, K * B], I8)        # x^T int8
            xbt = big.tile([N, K * B], BF16)      # x^T upcast to bf16
            ztt = big.tile([N, K * 128], BF16)    # Z sign stacks
            ytt = big.tile([128, K * M], FP16)    # scaled Y stacks
            tsb = [big.tile([128, B], FP16, name=f'tsb{i}') for i in range(3)]
            utt = sm.tile([UROWS, JLOC * M], FP16)
            rtt = sm.tile([UROWS, B], FP16)
            wz = sm.tile([128, 192], BF16)        # zeros for PE warmup
            nc.vector.memset(wz[:], 0.0)

            tp = [pp.tile([128, B], F32, name=f"tps{i}", tag=f"tps{i}")
                  for i in range(3)]
            outp = [pp.tile([128, B], F32, name=f"op{i}", tag=f"op{i}")
                    for i in range(4)]
            wps = pp.tile([128, 64], F32, name="wps", tag="wps")

            # Phase A: k-ordered streaming.  x^T int8 on the sync HWDGE
            # ring; Z/Y stacks on the scalar ring; correction matrices
            # first (they open the PSUM accumulation groups).  Slim dummy
            # matmuls paced by the first DMA warm the PE HAM clock.
            nc.gpsimd.dma_start(out=utt[:], in_=u_d[:])
            nc.gpsimd.dma_start(out=rtt[:], in_=r_d[:])
            k0 = 0
            for ci, nk in enumerate(CHUNKS):
                dma = nc.sync.dma_start(out=xi8[:, k0 * B:(k0 + nk) * B],
                                        in_=xt_d[:, k0 * B:(k0 + nk) * B])
                nc.scalar.dma_start(out=ztt[:, k0 * 128:(k0 + nk) * 128],
                                    in_=z_d[:, k0 * 128:(k0 + nk) * 128])
                nc.scalar.dma_start(out=ytt[:, k0 * M:(k0 + nk) * M],
                                    in_=y_d[:, k0 * M:(k0 + nk) * M])
                if ci == 0:
                    for w in range(WARMUP):
                        mm = nc.tensor.matmul(
                            wps[:], lhsT=wz[:, 0:128],
                            rhs=wz[:, 128:192], start=True, stop=True)
                        add_dep_helper(mm.ins, dma.ins,
                                       reason="pace PE warmup with x DMA")
                k0 += nk

            # x upcast int8 -> bf16 on DVE (the scalar engine faults on
            # int8 input; gpsimd is ~10x too slow)
            def upcast(k):
                nc.vector.tensor_copy(out=xbt[:, k * B:(k + 1) * B],
                                      in_=xi8[:, k * B:(k + 1) * B])

            UPLOOK = 6           # upcast lookahead inside the main k loop
            for k in range(UPLOOK):
                upcast(k)

            # Phase B: corrections open the output accumulation groups.
            for j in range(JLOC):
                nc.tensor.matmul(
                    outp[j][:], lhsT=utt[:, j * M:(j + 1) * M], rhs=rtt[:],
                    start=True, stop=False)

            def s1(k):
                nc.tensor.matmul(
                    tp[k % 3][:], lhsT=ztt[:, k * 128:(k + 1) * 128],
                    rhs=xbt[:, k * B:(k + 1) * B], start=True, stop=True)

            def tcopy(k):
                t = tsb[k % 3]
                nc.vector.tensor_copy(out=t[:, 0:128], in_=tp[k % 3][:, 0:128])
                nc.scalar.copy(t[:, 128:512], tp[k % 3][:, 128:512])

            def s2(k):
                for j in range(JLOC):
                    nc.tensor.matmul(
                        outp[j][:],
                        lhsT=ytt[32 * j:32 * (j + 1), k * M:(k + 1) * M],
                        rhs=tsb[k % 3][32 * j:32 * (j + 1), :],
                        start=False, stop=(k == K - 1),
                        tile_position=(32 * j, 0))

            s1(0)
            tcopy(0)
            s1(1)
            tcopy(1)
            for k in range(2, K):
                s1(k)
                s2(k - 2)
                tcopy(k)
                if k - 2 + UPLOOK < K:
                    upcast(k - 2 + UPLOOK)
            s2(K - 2)
            s2(K - 1)

            # Phase C: PSUM -> SBUF bf16 (split engines), DMA out rows
            # (j,m); the host transposes to [b, (j,m)].
            for j in range(JLOC):
                o = ot.tile([128, B], BF16)
                if j % 2 == 0:
                    nc.scalar.copy(o[:], outp[j][:])
                else:
                    nc.vector.tensor_copy(out=o[:], in_=outp[j][:])
                eng = nc.sync if j % 2 == 0 else nc.scalar
                eng.dma_start(out=out_d[j * 128:(j + 1) * 128, :], in_=o[:])
    return nc


def _prepare(inputs):
    x = np.asarray(inputs["input"], dtype=np.float32)
    Ys = np.asarray(inputs["Y_sign"], np.float32)
    Zs = np.asarray(inputs["Z_sign"], np.float32)
    ysc = np.asarray(inputs["Y_scale"], np.float32)[..., 0, 0]
    zsc = np.asarray(inputs["Z_scale"], np.float32)[..., 0, 0]
    A = np.asarray(inputs["A"], np.float32)
    bias = np.asarray(inputs["bias"], np.float32)
    a0, a1, a2, a3 = A[..., 0], A[..., 1], A[..., 2], A[..., 3]

    # activation quantization on host (exact global max/min, RNE round)
    act_scale = max((float(x.max()) - float(x.min())) / (2.0 * QMAX), 1e-8)
    Xi = np.clip(np.round(x.reshape(B, K * N) / act_scale), -QMAX, QMAX)
    Xkn = Xi.reshape(B, K, N)
    Sx = Xkn.sum(-1)                                   # [B,K] (exact ints)

    c = a0 * ysc * zsc * act_scale                     # [P,J,K]
    B2 = np.einsum('pjk,pjkm->jkm', a1 * ysc, Ys.sum(-1)) * act_scale
    C3 = np.einsum('pjk,pjkn->jkn', a2 * zsc, Zs.sum(-2))
    out3 = np.einsum('bkn,jkn->bj', Xkn, C3) * act_scale
    out4 = (Sx @ a3.sum(0).T) * act_scale              # [B,J]
    corr34 = out3 + out4

    xt8 = np.ascontiguousarray(
        Xi.reshape(B, K, N).transpose(2, 1, 0).reshape(N, K * B)).astype(
            np.int8)

    in_maps = []
    for cid in range(NCORES):
        jsl = slice(cid * JLOC, (cid + 1) * JLOC)
        zt = np.ascontiguousarray(
            Zs[:, jsl].transpose(4, 2, 1, 0, 3).reshape(N, K * 128)).astype(
                ml_dtypes.bfloat16)                    # [n,(k,j,p,l)]
        yt = np.ascontiguousarray(
            (c[:, jsl, :, None, None] * Ys[:, jsl]).transpose(
                1, 0, 4, 2, 3).reshape(128, K * M)).astype(np.float16)
        ut = np.zeros((UROWS, JLOC * M), np.float16)
        rt = np.zeros((UROWS, B), np.float16)
        for j in range(JLOC):
            jg = cid * JLOC + j
            ut[0:32, j * M:(j + 1) * M] = B2[jg].astype(np.float16)
            ut[32 + j, j * M:(j + 1) * M] = 1.0
            ut[36, j * M:(j + 1) * M] = bias[jg * M:(jg + 1) * M].astype(
                np.float16)
            rt[32 + j] = corr34[:, jg].astype(np.float16)
        rt[0:32] = Sx.T.astype(np.float16)
        rt[36] = 1.0
        in_maps.append({"xt8": xt8, "zt": zt, "yt": yt,
                        "ut": ut, "rt": rt})
    return in_maps


def _run(inputs, trace=False):
    if "nc" not in _CACHE:
        nc = _build_bass()
        nc.finalize()          # run bacc passes (reg alloc, wait splitting)
        _CACHE["nc"] = nc
    nc = _CACHE["nc"]
    in_maps = _prepare(inputs)
    res = run_bass_kernel_spmd(nc, in_maps, list(range(NCORES)), trace=trace)
    out = np.concatenate(
        [res.results[c]["out"].astype(np.float32).T for c in range(NCORES)],
        axis=1)
    out = out.reshape(1, B, J * M)
    return out, res


def kernel(**inputs) -> np.ndarray:
    out, _ = _run(inputs, trace=False)
    return out


# revision 17
# speedup vs baseline: 1.4459x; 1.0208x over previous
"""BQQ linear inference kernel for 8 Trainium2 NeuronCores.

Factorized form: after (host-side) activation quantization the op is linear
in X_int, and each (j,k) weight block is rank-32 plus rank-2 corrections:

  out[b, (j,m)] = act_scale * sum_k [ Yc_jk^T (Z_jk X_k^T) ]{m,b}  (main)
               + corrections(Sx, out3, out4) + bias

Device kernel per core (j-sharded, 4 of 32 j-blocks):
  stage0  corrections+bias enter PSUM as one 37-row matmul per j block
          (rows: 32 Sx terms, 4 per-j scalars, 1 bias row).
  stage1  T_k[(j,p,l), b] = Zstack_k^T @ X_k^T   (one 128x512 MM per k)
  copy    T_k PSUM -> SBUF fp16, split across vector/scalar engines
  stage2  outp_j[m, b] += Yc_jk^T @ T_k[32j:32j+32]  (4 row-tiled rank-32
          MMs per k, tile_position=(32j,0), software-pipelined one k behind
          stage1 so the PE never waits on the copies)
  out     PSUM -> SBUF bf16, DMA out as [jm, b]; host transposes.

x ships int8 (2MB) and is upcast to bf16 on the otherwise idle gpsimd
engine.  Per-core HBM traffic ~5.6 MB; PE streams ~half the dense GEMM.
"""

import numpy as np
import ml_dtypes

import concourse.bass as bass
import concourse.bacc as bacc
import concourse.mybir as mybir
from concourse.tile import TileContext
from concourse.tile_rust import add_dep_helper
from concourse.bass_utils import run_bass_kernel_spmd

F32 = mybir.dt.float32
BF16 = mybir.dt.bfloat16
FP16 = mybir.dt.float16
I8 = mybir.dt.int8

P_, J, K, M, L, N = 2, 32, 32, 128, 16, 128
B = 512                  # tokens
NCORES = 8
JLOC = J // NCORES       # 4 j-blocks per core
CPJ = JLOC * M           # 512 output rows per core ((j,m) major)
QMAX = 127.0
UROWS = 37               # 32 Sx rows + 4 corr rows + 1 bias row
HEADK = 6                # first k-slices shipped as ready bf16 (no upcast)
CHUNKS_H = [2, 4]        # bf16 head chunks (k-slices)
CHUNKS_I = [2, 4, 4, 4, 4, 4, 4]   # int8 chunks covering k = HEADK..31
WARMUP = 64

_CACHE = {}


def _build_bass():
    nc = bacc.Bacc()
    xt_d = nc.declare_dram_parameter("xt8", [N, K * B], I8, isOutput=False)
    xh_d = nc.declare_dram_parameter("xth", [N, HEADK * B], BF16,
                                     isOutput=False)
    z_d = nc.declare_dram_parameter("zt", [N, K * 128], BF16, isOutput=False)
    y_d = nc.declare_dram_parameter("yt", [128, K * M], FP16, isOutput=False)
    u_d = nc.declare_dram_parameter("ut", [UROWS, JLOC * M], FP16,
                                    isOutput=False)
    r_d = nc.declare_dram_parameter("rt", [UROWS, B], FP16, isOutput=False)
    out_d = nc.declare_dram_parameter("out", [CPJ, B], BF16, isOutput=True)

    with TileContext(nc) as tc:
        with tc.tile_pool(name="big", bufs=1) as big, \
             tc.tile_pool(name="sm", bufs=1) as sm, \
             tc.tile_pool(name="ot", bufs=4) as ot, \
             tc.tile_pool(name="psum", bufs=1, space="PSUM") as pp:
            xi8 = big.tile([N, K * B], I8)        # x^T int8
            xbt = big.tile([N, K * B], BF16)      # x^T upcast to bf16
            ztt = big.tile([N, K * 128], BF16)    # Z sign stacks
            ytt = big.tile([128, K * M], FP16)    # scaled Y stacks
            tsb = [big.tile([128, B], FP16, name=f'tsb{i}') for i in range(3)]
            utt = sm.tile([UROWS, JLOC * M], FP16)
            rtt = sm.tile([UROWS, B], FP16)
            wz = sm.tile([128, 192], BF16)        # zeros for PE warmup
            nc.vector.memset(wz[:], 0.0)

            tp = [pp.tile([128, B], F32, name=f"tps{i}", tag=f"tps{i}")
                  for i in range(3)]
            outp = [pp.tile([128, B], F32, name=f"op{i}", tag=f"op{i}")
                    for i in range(4)]
            wps = pp.tile([128, 64], F32, name="wps", tag="wps")

            # Phase A: k-ordered streaming.  x^T int8 on the sync HWDGE
            # ring; Z/Y stacks on the scalar ring; correction matrices
            # first (they open the PSUM accumulation groups).  Slim dummy
            # matmuls paced by the first DMA warm the PE HAM clock.
            nc.gpsimd.dma_start(out=utt[:], in_=u_d[:])
            nc.gpsimd.dma_start(out=rtt[:], in_=r_d[:])
            k0 = 0
            first = None
            for nk in CHUNKS_H:
                dma = nc.sync.dma_start(out=xbt[:, k0 * B:(k0 + nk) * B],
                                        in_=xh_d[:, k0 * B:(k0 + nk) * B])
                first = first or dma
                nc.scalar.dma_start(out=ztt[:, k0 * 128:(k0 + nk) * 128],
                                    in_=z_d[:, k0 * 128:(k0 + nk) * 128])
                nc.scalar.dma_start(out=ytt[:, k0 * M:(k0 + nk) * M],
                                    in_=y_d[:, k0 * M:(k0 + nk) * M])
                k0 += nk
            for w in range(WARMUP):
                mm = nc.tensor.matmul(
                    wps[:], lhsT=wz[:, 0:128],
                    rhs=wz[:, 128:192], start=True, stop=True)
                add_dep_helper(mm.ins, first.ins,
                               reason="pace PE warmup with x DMA")
            for nk in CHUNKS_I:
                nc.sync.dma_start(out=xi8[:, k0 * B:(k0 + nk) * B],
                                  in_=xt_d[:, k0 * B:(k0 + nk) * B])
                nc.scalar.dma_start(out=ztt[:, k0 * 128:(k0 + nk) * 128],
                                    in_=z_d[:, k0 * 128:(k0 + nk) * 128])
                nc.scalar.dma_start(out=ytt[:, k0 * M:(k0 + nk) * M],
                                    in_=y_d[:, k0 * M:(k0 + nk) * M])
                k0 += nk

            # x upcast int8 -> bf16 on DVE for k >= HEADK (the scalar
            # engine faults on int8 input; gpsimd is ~10x too slow)
            def upcast(k):
                nc.vector.tensor_copy(out=xbt[:, k * B:(k + 1) * B],
                                      in_=xi8[:, k * B:(k + 1) * B])

            # Phase B: corrections open the output accumulation groups.
            for j in range(JLOC):
                nc.tensor.matmul(
                    outp[j][:], lhsT=utt[:, j * M:(j + 1) * M], rhs=rtt[:],
                    start=True, stop=False)

            def s1(k):
                nc.tensor.matmul(
                    tp[k % 3][:], lhsT=ztt[:, k * 128:(k + 1) * 128],
                    rhs=xbt[:, k * B:(k + 1) * B], start=True, stop=True)

            def tcopy(k):
                t = tsb[k % 3]
                nc.vector.tensor_copy(out=t[:, 0:128], in_=tp[k % 3][:, 0:128])
                nc.scalar.copy(t[:, 128:512], tp[k % 3][:, 128:512])

            def s2(k):
                for j in range(JLOC):
                    nc.tensor.matmul(
                        outp[j][:],
                        lhsT=ytt[32 * j:32 * (j + 1), k * M:(k + 1) * M],
                        rhs=tsb[k % 3][32 * j:32 * (j + 1), :],
                        start=False, stop=(k == K - 1),
                        tile_position=(32 * j, 0))

            s1(0)
            tcopy(0)
            s1(1)
            tcopy(1)
            for k in range(2, K):
                s1(k)
                if k < 20:       # hold the HAM clock warm through startup
                    nc.tensor.matmul(wps[:], lhsT=wz[:, 0:128],
                                     rhs=wz[:, 128:192], start=True,
                                     stop=True)
                s2(k - 2)
                tcopy(k)
                ku = k + 4
                if HEADK <= ku < K:
                    upcast(ku)
            s2(K - 2)
            s2(K - 1)

            # Phase C: PSUM -> SBUF bf16 (split engines), DMA out rows
            # (j,m); the host transposes to [b, (j,m)].
            for j in range(JLOC):
                o = ot.tile([128, B], BF16)
                if j % 2 == 0:
                    nc.scalar.copy(o[:], outp[j][:])
                else:
                    nc.vector.tensor_copy(out=o[:], in_=outp[j][:])
                eng = nc.sync if j % 2 == 0 else nc.scalar
                eng.dma_start(out=out_d[j * 128:(j + 1) * 128, :], in_=o[:])
    return nc


def _prepare(inputs):
    x = np.asarray(inputs["input"], dtype=np.float32)
    Ys = np.asarray(inputs["Y_sign"], np.float32)
    Zs = np.asarray(inputs["Z_sign"], np.float32)
    ysc = np.asarray(inputs["Y_scale"], np.float32)[..., 0, 0]
    zsc = np.asarray(inputs["Z_scale"], np.float32)[..., 0, 0]
    A = np.asarray(inputs["A"], np.float32)
    bias = np.asarray(inputs["bias"], np.float32)
    a0, a1, a2, a3 = A[..., 0], A[..., 1], A[..., 2], A[..., 3]

    # activation quantization on host (exact global max/min, RNE round)
    act_scale = max((float(x.max()) - float(x.min())) / (2.0 * QMAX), 1e-8)
    Xi = np.clip(np.round(x.reshape(B, K * N) / act_scale), -QMAX, QMAX)
    Xkn = Xi.reshape(B, K, N)
    Sx = Xkn.sum(-1)                                   # [B,K] (exact ints)

    c = a0 * ysc * zsc * act_scale                     # [P,J,K]
    B2 = np.einsum('pjk,pjkm->jkm', a1 * ysc, Ys.sum(-1)) * act_scale
    C3 = np.einsum('pjk,pjkn->jkn', a2 * zsc, Zs.sum(-2))
    out3 = np.einsum('bkn,jkn->bj', Xkn, C3) * act_scale
    out4 = (Sx @ a3.sum(0).T) * act_scale              # [B,J]
    corr34 = out3 + out4

    xtT = np.ascontiguousarray(
        Xi.reshape(B, K, N).transpose(2, 1, 0).reshape(N, K * B))
    xt8 = xtT.astype(np.int8)
    xth = np.ascontiguousarray(xtT[:, 0:HEADK * B]).astype(ml_dtypes.bfloat16)

    in_maps = []
    for cid in range(NCORES):
        jsl = slice(cid * JLOC, (cid + 1) * JLOC)
        zt = np.ascontiguousarray(
            Zs[:, jsl].transpose(4, 2, 1, 0, 3).reshape(N, K * 128)).astype(
                ml_dtypes.bfloat16)                    # [n,(k,j,p,l)]
        yt = np.ascontiguousarray(
            (c[:, jsl, :, None, None] * Ys[:, jsl]).transpose(
                1, 0, 4, 2, 3).reshape(128, K * M)).astype(np.float16)
        ut = np.zeros((UROWS, JLOC * M), np.float16)
        rt = np.zeros((UROWS, B), np.float16)
        for j in range(JLOC):
            jg = cid * JLOC + j
            ut[0:32, j * M:(j + 1) * M] = B2[jg].astype(np.float16)
            ut[32 + j, j * M:(j + 1) * M] = 1.0
            ut[36, j * M:(j + 1) * M] = bias[jg * M:(jg + 1) * M].astype(
                np.float16)
            rt[32 + j] = corr34[:, jg].astype(np.float16)
        rt[0:32] = Sx.T.astype(np.float16)
        rt[36] = 1.0
        in_maps.append({"xt8": xt8, "xth": xth, "zt": zt, "yt": yt,
                        "ut": ut, "rt": rt})
    return in_maps


def _run(inputs, trace=False):
    if "nc" not in _CACHE:
        nc = _build_bass()
        nc.finalize()          # run bacc passes (reg alloc, wait splitting)
        _CACHE["nc"] = nc
    nc = _CACHE["nc"]
    in_maps = _prepare(inputs)
    res = run_bass_kernel_spmd(nc, in_maps, list(range(NCORES)), trace=trace)
    out = np.concatenate(
        [res.results[c]["out"].astype(np.float32).T for c in range(NCORES)],
        axis=1)
    out = out.reshape(1, B, J * M)
    return out, res


def kernel(**inputs) -> np.ndarray:
    out, _ = _run(inputs, trace=False)
    return out


# revision 18
# speedup vs baseline: 1.5301x; 1.0582x over previous
"""BQQ linear inference kernel for 8 Trainium2 NeuronCores.

Factorized form: after (host-side) activation quantization the op is linear
in X_int, and each (j,k) weight block is rank-32 plus rank-2 corrections:

  out[b, (j,m)] = act_scale * sum_k [ Yc_jk^T (Z_jk X_k^T) ]{m,b}  (main)
               + corrections(Sx, out3, out4) + bias

Device kernel per core (j-sharded, 4 of 32 j-blocks):
  stage0  corrections+bias enter PSUM as one 37-row matmul per j block
          (rows: 32 Sx terms, 4 per-j scalars, 1 bias row).
  stage1  T_k[(j,p,l), b] = Zstack_k^T @ X_k^T   (one 128x512 MM per k)
  copy    T_k PSUM -> SBUF fp16, split across vector/scalar engines
  stage2  outp_j[m, b] += Yc_jk^T @ T_k[32j:32j+32]  (4 row-tiled rank-32
          MMs per k, tile_position=(32j,0), software-pipelined one k behind
          stage1 so the PE never waits on the copies)
  out     PSUM -> SBUF bf16, DMA out as [jm, b]; host transposes.

x ships int8 (2MB) and is upcast to bf16 on the otherwise idle gpsimd
engine.  Per-core HBM traffic ~5.6 MB; PE streams ~half the dense GEMM.
"""

import numpy as np
import ml_dtypes

import concourse.bass as bass
import concourse.bacc as bacc
import concourse.mybir as mybir
from concourse.tile import TileContext
from concourse.tile_rust import add_dep_helper
from concourse.bass_utils import run_bass_kernel_spmd

F32 = mybir.dt.float32
BF16 = mybir.dt.bfloat16
FP16 = mybir.dt.float16
I8 = mybir.dt.int8

P_, J, K, M, L, N = 2, 32, 32, 128, 16, 128
B = 512                  # tokens
NCORES = 8
JLOC = J // NCORES       # 4 j-blocks per core
CPJ = JLOC * M           # 512 output rows per core ((j,m) major)
QMAX = 127.0
UROWS = 37               # 32 Sx rows + 4 corr rows + 1 bias row
HEADK = 6                # first k-slices shipped as ready bf16 (no upcast)
WARMUP = 56

_CACHE = {}


def _build_bass():
    nc = bacc.Bacc()
    xt_d = nc.declare_dram_parameter("xt8", [N, K * B], I8, isOutput=False)
    xh_d = nc.declare_dram_parameter("xth", [N, HEADK * B], BF16,
                                     isOutput=False)
    z_d = nc.declare_dram_parameter("zt", [N, K * 128], BF16, isOutput=False)
    y_d = nc.declare_dram_parameter("yt", [128, K * M], FP16, isOutput=False)
    u_d = nc.declare_dram_parameter("ut", [UROWS, JLOC * M], FP16,
                                    isOutput=False)
    r_d = nc.declare_dram_parameter("rt", [UROWS, B], FP16, isOutput=False)
    out_d = nc.declare_dram_parameter("out", [CPJ, B], BF16, isOutput=True)

    with TileContext(nc) as tc:
        with tc.tile_pool(name="big", bufs=1) as big, \
             tc.tile_pool(name="sm", bufs=1) as sm, \
             tc.tile_pool(name="ot", bufs=4) as ot, \
             tc.tile_pool(name="psum", bufs=1, space="PSUM") as pp:
            xi8 = big.tile([N, K * B], I8)        # x^T int8
            xbt = big.tile([N, K * B], BF16)      # x^T upcast to bf16
            ztt = big.tile([N, K * 128], BF16)    # Z sign stacks
            ytt = big.tile([128, K * M], FP16)    # scaled Y stacks
            tsb = [big.tile([128, B], FP16, name=f'tsb{i}') for i in range(3)]
            utt = sm.tile([UROWS, JLOC * M], FP16)
            rtt = sm.tile([UROWS, B], FP16)
            wz = sm.tile([128, 192], BF16)        # zeros for PE warmup
            wzms = nc.vector.memset(wz[:], 0.0)

            tp = [pp.tile([128, B], F32, name=f"tps{i}", tag=f"tps{i}")
                  for i in range(3)]
            outp = [pp.tile([128, B], F32, name=f"op{i}", tag=f"op{i}")
                    for i in range(4)]
            wps = pp.tile([128, 64], F32, name="wps", tag="wps")

            # Phase A: k-ordered streaming.  x^T int8 on the sync HWDGE
            # ring; Z/Y stacks on the scalar ring; correction matrices
            # first (they open the PSUM accumulation groups).  Slim dummy
            # matmuls paced by the first DMA warm the PE HAM clock.
            nc.gpsimd.dma_start(out=utt[:], in_=u_d[:])
            nc.gpsimd.dma_start(out=rtt[:], in_=r_d[:])
            # All input DMAs issue from the (otherwise idle) sync engine in
            # one k-ordered sequence; each trigger costs ~650ns of engine
            # time, so keeping them off vector/scalar matters.
            def dx_h(ka, kb):      # bf16 head slices of x
                nc.sync.dma_start(out=xbt[:, ka * B:kb * B],
                                  in_=xh_d[:, ka * B:kb * B])
            def dx_i(ka, kb):      # int8 slices of x
                nc.sync.dma_start(out=xi8[:, ka * B:kb * B],
                                  in_=xt_d[:, ka * B:kb * B])
            def dz(ka, kb):
                nc.sync.dma_start(out=ztt[:, ka * 128:kb * 128],
                                  in_=z_d[:, ka * 128:kb * 128])
            def dy(ka, kb):
                nc.sync.dma_start(out=ytt[:, ka * M:kb * M],
                                  in_=y_d[:, ka * M:kb * M])
            dx_h(0, 2); dz(0, 8); dy(0, 8); dx_h(2, 6)
            dx_i(6, 10); dz(8, 16); dy(8, 16); dx_i(10, 14)
            dx_i(14, 18); dz(16, 24); dy(16, 24); dx_i(18, 22)
            dx_i(22, 26); dz(24, 32); dy(24, 32); dx_i(26, 32)
            # PE warmup paced off the wz memset so it runs during the DMA
            # wait and hands the HAM clock over warm.
            for w in range(WARMUP):
                mm = nc.tensor.matmul(
                    wps[:], lhsT=wz[:, 0:128],
                    rhs=wz[:, 128:192], start=True, stop=True)
                add_dep_helper(mm.ins, wzms.ins,
                               reason="pace PE warmup after wz memset")

            # x upcast int8 -> bf16 on DVE for k >= HEADK (the scalar
            # engine faults on int8 input; gpsimd is ~10x too slow)
            def upcast(k):
                nc.vector.tensor_copy(out=xbt[:, k * B:(k + 1) * B],
                                      in_=xi8[:, k * B:(k + 1) * B])

            # Phase B: corrections open the output accumulation groups.
            for j in range(JLOC):
                nc.tensor.matmul(
                    outp[j][:], lhsT=utt[:, j * M:(j + 1) * M], rhs=rtt[:],
                    start=True, stop=False)

            def s1(k):
                nc.tensor.matmul(
                    tp[k % 3][:], lhsT=ztt[:, k * 128:(k + 1) * 128],
                    rhs=xbt[:, k * B:(k + 1) * B], start=True, stop=True)

            def tcopy(k):
                t = tsb[k % 3]
                nc.vector.tensor_copy(out=t[:, 0:128], in_=tp[k % 3][:, 0:128])
                nc.scalar.copy(t[:, 128:512], tp[k % 3][:, 128:512])

            def s2(k):
                for j in range(JLOC):
                    nc.tensor.matmul(
                        outp[j][:],
                        lhsT=ytt[32 * j:32 * (j + 1), k * M:(k + 1) * M],
                        rhs=tsb[k % 3][32 * j:32 * (j + 1), :],
                        start=False, stop=(k == K - 1),
                        tile_position=(32 * j, 0))

            s1(0)
            tcopy(0)
            s1(1)
            tcopy(1)
            for k in range(2, K):
                s1(k)
                if k < 20:       # hold the HAM clock warm through startup
                    nc.tensor.matmul(wps[:], lhsT=wz[:, 0:128],
                                     rhs=wz[:, 128:192], start=True,
                                     stop=True)
                s2(k - 2)
                tcopy(k)
                ku = k + 4
                if HEADK <= ku < K:
                    upcast(ku)
            s2(K - 2)
            s2(K - 1)

            # Phase C: PSUM -> SBUF bf16 (split engines), DMA out rows
            # (j,m); the host transposes to [b, (j,m)].
            for j in range(JLOC):
                o = ot.tile([128, B], BF16)
                if j % 2 == 0:
                    nc.scalar.copy(o[:], outp[j][:])
                else:
                    nc.vector.tensor_copy(out=o[:], in_=outp[j][:])
                eng = nc.sync if j % 2 == 0 else nc.scalar
                eng.dma_start(out=out_d[j * 128:(j + 1) * 128, :], in_=o[:])
    return nc


def _prepare(inputs):
    x = np.asarray(inputs["input"], dtype=np.float32)
    Ys = np.asarray(inputs["Y_sign"], np.float32)
    Zs = np.asarray(inputs["Z_sign"], np.float32)
    ysc = np.asarray(inputs["Y_scale"], np.float32)[..., 0, 0]
    zsc = np.asarray(inputs["Z_scale"], np.float32)[..., 0, 0]
    A = np.asarray(inputs["A"], np.float32)
    bias = np.asarray(inputs["bias"], np.float32)
    a0, a1, a2, a3 = A[..., 0], A[..., 1], A[..., 2], A[..., 3]

    # activation quantization on host (exact global max/min, RNE round)
    act_scale = max((float(x.max()) - float(x.min())) / (2.0 * QMAX), 1e-8)
    Xi = np.clip(np.round(x.reshape(B, K * N) / act_scale), -QMAX, QMAX)
    Xkn = Xi.reshape(B, K, N)
    Sx = Xkn.sum(-1)                                   # [B,K] (exact ints)

    c = a0 * ysc * zsc * act_scale                     # [P,J,K]
    B2 = np.einsum('pjk,pjkm->jkm', a1 * ysc, Ys.sum(-1)) * act_scale
    C3 = np.einsum('pjk,pjkn->jkn', a2 * zsc, Zs.sum(-2))
    out3 = np.einsum('bkn,jkn->bj', Xkn, C3) * act_scale
    out4 = (Sx @ a3.sum(0).T) * act_scale              # [B,J]
    corr34 = out3 + out4

    xtT = np.ascontiguousarray(
        Xi.reshape(B, K, N).transpose(2, 1, 0).reshape(N, K * B))
    xt8 = xtT.astype(np.int8)
    xth = np.ascontiguousarray(xtT[:, 0:HEADK * B]).astype(ml_dtypes.bfloat16)

    in_maps = []
    for cid in range(NCORES):
        jsl = slice(cid * JLOC, (cid + 1) * JLOC)
        zt = np.ascontiguousarray(
            Zs[:, jsl].transpose(4, 2, 1, 0, 3).reshape(N, K * 128)).astype(
                ml_dtypes.bfloat16)                    # [n,(k,j,p,l)]
        yt = np.ascontiguousarray(
            (c[:, jsl, :, None, None] * Ys[:, jsl]).transpose(
                1, 0, 4, 2, 3).reshape(128, K * M)).astype(np.float16)
        ut = np.zeros((UROWS, JLOC * M), np.float16)
        rt = np.zeros((UROWS, B), np.float16)
        for j in range(JLOC):
            jg = cid * JLOC + j
            ut[0:32, j * M:(j + 1) * M] = B2[jg].astype(np.float16)
            ut[32 + j, j * M:(j + 1) * M] = 1.0
            ut[36, j * M:(j + 1) * M] = bias[jg * M:(jg + 1) * M].astype(
                np.float16)
            rt[32 + j] = corr34[:, jg].astype(np.float16)
        rt[0:32] = Sx.T.astype(np.float16)
        rt[36] = 1.0
        in_maps.append({"xt8": xt8, "xth": xth, "zt": zt, "yt": yt,
                        "ut": ut, "rt": rt})
    return in_maps


def _run(inputs, trace=False):
    if "nc" not in _CACHE:
        nc = _build_bass()
        nc.finalize()          # run bacc passes (reg alloc, wait splitting)
        _CACHE["nc"] = nc
    nc = _CACHE["nc"]
    in_maps = _prepare(inputs)
    res = run_bass_kernel_spmd(nc, in_maps, list(range(NCORES)), trace=trace)
    out = np.concatenate(
        [res.results[c]["out"].astype(np.float32).T for c in range(NCORES)],
        axis=1)
    out = out.reshape(1, B, J * M)
    return out, res


def kernel(**inputs) -> np.ndarray:
    out, _ = _run(inputs, trace=False)
    return out


# revision 19
# speedup vs baseline: 1.6181x; 1.0575x over previous
"""BQQ linear inference kernel for 8 Trainium2 NeuronCores.

Factorized form: after (host-side) activation quantization the op is linear
in X_int, and each (j,k) weight block is rank-32 plus rank-2 corrections:

  out[b, (j,m)] = act_scale * sum_k [ Yc_jk^T (Z_jk X_k^T) ]{m,b}  (main)
               + corrections(Sx, out3, out4) + bias

Device kernel per core (j-sharded, 4 of 32 j-blocks):
  stage0  corrections+bias enter PSUM as one 37-row matmul per j block
          (rows: 32 Sx terms, 4 per-j scalars, 1 bias row).
  stage1  T_k[(j,p,l), b] = Zstack_k^T @ X_k^T   (one 128x512 MM per k)
  copy    T_k PSUM -> SBUF fp16, split across vector/scalar engines
  stage2  outp_j[m, b] += Yc_jk^T @ T_k[32j:32j+32]  (4 row-tiled rank-32
          MMs per k, tile_position=(32j,0), software-pipelined one k behind
          stage1 so the PE never waits on the copies)
  out     PSUM -> SBUF bf16, DMA out as [jm, b]; host transposes.

x ships int8 (2MB) and is upcast to bf16 on the otherwise idle gpsimd
engine.  Per-core HBM traffic ~5.6 MB; PE streams ~half the dense GEMM.
"""

import numpy as np
import ml_dtypes

import concourse.bass as bass
import concourse.bacc as bacc
import concourse.mybir as mybir
from concourse.tile import TileContext
from concourse.tile_rust import add_dep_helper
from concourse.bass_utils import run_bass_kernel_spmd

F32 = mybir.dt.float32
BF16 = mybir.dt.bfloat16
FP16 = mybir.dt.float16
I8 = mybir.dt.int8

P_, J, K, M, L, N = 2, 32, 32, 128, 16, 128
B = 512                  # tokens
NCORES = 8
JLOC = J // NCORES       # 4 j-blocks per core
CPJ = JLOC * M           # 512 output rows per core ((j,m) major)
QMAX = 127.0
UROWS = 37               # 32 Sx rows + 4 corr rows + 1 bias row
HEADK = 6                # first k-slices shipped as ready bf16 (no upcast)
WARMUP = 72

_CACHE = {}


def _build_bass():
    nc = bacc.Bacc()
    xt_d = nc.declare_dram_parameter("xt8", [N, K * B], I8, isOutput=False)
    xh_d = nc.declare_dram_parameter("xth", [N, HEADK * B], BF16,
                                     isOutput=False)
    z_d = nc.declare_dram_parameter("zt", [N, K * 128], BF16, isOutput=False)
    y_d = nc.declare_dram_parameter("yt", [128, K * M], FP16, isOutput=False)
    u_d = nc.declare_dram_parameter("ut", [UROWS, JLOC * M], FP16,
                                    isOutput=False)
    r_d = nc.declare_dram_parameter("rt", [UROWS, B], FP16, isOutput=False)
    out_d = nc.declare_dram_parameter("out", [CPJ, B], BF16, isOutput=True)

    with TileContext(nc) as tc:
        with tc.tile_pool(name="big", bufs=1) as big, \
             tc.tile_pool(name="sm", bufs=1) as sm, \
             tc.tile_pool(name="ot", bufs=4) as ot, \
             tc.tile_pool(name="psum", bufs=1, space="PSUM") as pp:
            xi8 = big.tile([N, K * B], I8)        # x^T int8
            xbt = big.tile([N, K * B], BF16)      # x^T upcast to bf16
            ztt = big.tile([N, K * 128], BF16)    # Z sign stacks
            ytt = big.tile([128, K * M], FP16)    # scaled Y stacks
            tsb = [big.tile([128, B], FP16, name=f'tsb{i}') for i in range(3)]
            utt = sm.tile([UROWS, JLOC * M], FP16)
            rtt = sm.tile([UROWS, B], FP16)
            wz = sm.tile([128, 192], BF16)        # zeros for PE warmup
            wzms = nc.vector.memset(wz[:], 0.0)

            tp = [pp.tile([128, B], F32, name=f"tps{i}", tag=f"tps{i}")
                  for i in range(3)]
            outp = [pp.tile([128, B], F32, name=f"op{i}", tag=f"op{i}")
                    for i in range(4)]
            wps = pp.tile([128, 64], F32, name="wps", tag="wps")

            # Phase A: k-ordered streaming.  x^T int8 on the sync HWDGE
            # ring; Z/Y stacks on the scalar ring; correction matrices
            # first (they open the PSUM accumulation groups).  Slim dummy
            # matmuls paced by the first DMA warm the PE HAM clock.
            # All input DMAs issue from the (otherwise idle) sync engine in
            # one k-ordered sequence; each trigger costs ~650ns of engine
            # time, so keeping them off vector/scalar matters.
            def dx_h(ka, kb):      # bf16 head slices of x
                nc.sync.dma_start(out=xbt[:, ka * B:kb * B],
                                  in_=xh_d[:, ka * B:kb * B])
            def dx_i(ka, kb):      # int8 slices of x
                nc.sync.dma_start(out=xi8[:, ka * B:kb * B],
                                  in_=xt_d[:, ka * B:kb * B])
            def dz(ka, kb):
                nc.sync.dma_start(out=ztt[:, ka * 128:kb * 128],
                                  in_=z_d[:, ka * 128:kb * 128])
            def dy(ka, kb):
                nc.sync.dma_start(out=ytt[:, ka * M:kb * M],
                                  in_=y_d[:, ka * M:kb * M])
            nc.sync.dma_start(out=utt[:], in_=u_d[:])
            nc.sync.dma_start(out=rtt[:], in_=r_d[:])
            dx_h(0, 2); dz(0, 8); dy(0, 8); dx_h(2, 6)
            dx_i(6, 10); dz(8, 16); dy(8, 16); dx_i(10, 14)
            dx_i(14, 18); dz(16, 24); dy(16, 24); dx_i(18, 22)
            dx_i(22, 26); dz(24, 32); dy(24, 32); dx_i(26, 32)
            # PE warmup paced off the wz memset so it runs during the DMA
            # wait and hands the HAM clock over warm.
            for w in range(WARMUP):
                mm = nc.tensor.matmul(
                    wps[:], lhsT=wz[:, 0:128],
                    rhs=wz[:, 128:192], start=True, stop=True)
                add_dep_helper(mm.ins, wzms.ins,
                               reason="pace PE warmup after wz memset")

            # x upcast int8 -> bf16 on DVE for k >= HEADK (the scalar
            # engine faults on int8 input; gpsimd is ~10x too slow)
            def upcast(k):
                nc.vector.tensor_copy(out=xbt[:, k * B:(k + 1) * B],
                                      in_=xi8[:, k * B:(k + 1) * B])

            # Phase B: corrections open the output accumulation groups.
            for j in range(JLOC):
                nc.tensor.matmul(
                    outp[j][:], lhsT=utt[:, j * M:(j + 1) * M], rhs=rtt[:],
                    start=True, stop=False)

            def s1(k):
                nc.tensor.matmul(
                    tp[k % 3][:], lhsT=ztt[:, k * 128:(k + 1) * 128],
                    rhs=xbt[:, k * B:(k + 1) * B], start=True, stop=True)

            def tcopy(k):
                t = tsb[k % 3]
                nc.vector.tensor_copy(out=t[:, 0:128], in_=tp[k % 3][:, 0:128])
                nc.scalar.copy(t[:, 128:512], tp[k % 3][:, 128:512])

            def s2(k):
                for j in range(JLOC):
                    nc.tensor.matmul(
                        outp[j][:],
                        lhsT=ytt[32 * j:32 * (j + 1), k * M:(k + 1) * M],
                        rhs=tsb[k % 3][32 * j:32 * (j + 1), :],
                        start=False, stop=(k == K - 1),
                        tile_position=(32 * j, 0))

            s1(0)
            tcopy(0)
            s1(1)
            tcopy(1)
            for k in range(2, K):
                s1(k)
                s2(k - 2)
                tcopy(k)
                ku = k + 4
                if HEADK <= ku < K:
                    upcast(ku)
            s2(K - 2)
            s2(K - 1)

            # Phase C: PSUM -> SBUF bf16 (split engines), DMA out rows
            # (j,m); the host transposes to [b, (j,m)].
            for j in range(JLOC):
                o = ot.tile([128, B], BF16)
                if j % 2 == 0:
                    nc.scalar.copy(o[:], outp[j][:])
                else:
                    nc.vector.tensor_copy(out=o[:], in_=outp[j][:])
                eng = nc.sync if j % 2 == 0 else nc.scalar
                eng.dma_start(out=out_d[j * 128:(j + 1) * 128, :], in_=o[:])
    return nc


def _prepare(inputs):
    x = np.asarray(inputs["input"], dtype=np.float32)
    Ys = np.asarray(inputs["Y_sign"], np.float32)
    Zs = np.asarray(inputs["Z_sign"], np.float32)
    ysc = np.asarray(inputs["Y_scale"], np.float32)[..., 0, 0]
    zsc = np.asarray(inputs["Z_scale"], np.float32)[..., 0, 0]
    A = np.asarray(inputs["A"], np.float32)
    bias = np.asarray(inputs["bias"], np.float32)
    a0, a1, a2, a3 = A[..., 0], A[..., 1], A[..., 2], A[..., 3]

    # activation quantization on host (exact global max/min, RNE round)
    act_scale = max((float(x.max()) - float(x.min())) / (2.0 * QMAX), 1e-8)
    Xi = np.clip(np.round(x.reshape(B, K * N) / act_scale), -QMAX, QMAX)
    Xkn = Xi.reshape(B, K, N)
    Sx = Xkn.sum(-1)                                   # [B,K] (exact ints)

    c = a0 * ysc * zsc * act_scale                     # [P,J,K]
    B2 = np.einsum('pjk,pjkm->jkm', a1 * ysc, Ys.sum(-1)) * act_scale
    C3 = np.einsum('pjk,pjkn->jkn', a2 * zsc, Zs.sum(-2))
    out3 = np.einsum('bkn,jkn->bj', Xkn, C3) * act_scale
    out4 = (Sx @ a3.sum(0).T) * act_scale              # [B,J]
    corr34 = out3 + out4

    xtT = np.ascontiguousarray(
        Xi.reshape(B, K, N).transpose(2, 1, 0).reshape(N, K * B))
    xt8 = xtT.astype(np.int8)
    xth = np.ascontiguousarray(xtT[:, 0:HEADK * B]).astype(ml_dtypes.bfloat16)

    in_maps = []
    for cid in range(NCORES):
        jsl = slice(cid * JLOC, (cid + 1) * JLOC)
        zt = np.ascontiguousarray(
            Zs[:, jsl].transpose(4, 2, 1, 0, 3).reshape(N, K * 128)).astype(
                ml_dtypes.bfloat16)                    # [n,(k,j,p,l)]
        yt = np.ascontiguousarray(
            (c[:, jsl, :, None, None] * Ys[:, jsl]).transpose(
                1, 0, 4, 2, 3).reshape(128, K * M)).astype(np.float16)
        ut = np.zeros((UROWS, JLOC * M), np.float16)
        rt = np.zeros((UROWS, B), np.float16)
        for j in range(JLOC):
            jg = cid * JLOC + j
            ut[0:32, j * M:(j + 1) * M] = B2[jg].astype(np.float16)
            ut[32 + j, j * M:(j + 1) * M] = 1.0
            ut[36, j * M:(j + 1) * M] = bias[jg * M:(jg + 1) * M].astype(
                np.float16)
            rt[32 + j] = corr34[:, jg].astype(np.float16)
        rt[0:32] = Sx.T.astype(np.float16)
        rt[36] = 1.0
        in_maps.append({"xt8": xt8, "xth": xth, "zt": zt, "yt": yt,
                        "ut": ut, "rt": rt})
    return in_maps


def _run(inputs, trace=False):
    if "nc" not in _CACHE:
        nc = _build_bass()
        nc.finalize()          # run bacc passes (reg alloc, wait splitting)
        _CACHE["nc"] = nc
    nc = _CACHE["nc"]
    in_maps = _prepare(inputs)
    res = run_bass_kernel_spmd(nc, in_maps, list(range(NCORES)), trace=trace)
    out = np.concatenate(
        [res.results[c]["out"].astype(np.float32).T for c in range(NCORES)],
        axis=1)
    out = out.reshape(1, B, J * M)
    return out, res


def kernel(**inputs) -> np.ndarray:
    out, _ = _run(inputs, trace=False)
    return out
